# revision 1
# baseline (speedup 1.0000x reference)
"""Trainium2 kernel for nn_GATv5 (2-layer GATv2 + encoder MLP).

Structure exploited: with xc = concat(x, x1, x2) (x1,x2 are [N,1] GAT outputs),
the only heavy work is the fused matmul  x @ [Wl1 | Wr1 | enc_W1[:IN]]  — a
[10000, 9998] x [9998, 80] product (400MB of input traffic, ~16 GFLOP). That
runs on 8 NeuronCores, row-sharded (1250 rows/core, zero collectives). The
edge-softmax / segment ops (~5 MFLOP on 330k edges) run on host, as do the
tiny [N,8]x[8,1] and [N,64]x[64,32]x[32,1] tails.
"""

import sys
import numpy as np

sys.path.insert(0, "/opt/trn_rl_repo")

N = 10000
IN = 9998
E = 320000
H, C = 2, 4
NEG = 0.2
NCORES = 8
ROWS = N // NCORES          # 1250 rows of x per core
NK = 79                     # k-tiles of 128 over the padded contraction dim
KP = NK * 128               # 10112 (IN padded with zeros)
WCOLS = 80                  # 8 (Wl1) + 8 (Wr1) + 64 (enc_W1 cols)
ROW_SPLITS = [(0, 512), (512, 512), (1024, 226)]

_compiled = {}


def _build_module():
    from concourse import bacc, tile, mybir

    F32 = mybir.dt.float32
    BF16 = mybir.dt.bfloat16

    nc = bacc.Bacc(target_bir_lowering=False)
    x_t = nc.declare_dram_parameter("x_t", [KP, ROWS], BF16, isOutput=False)
    w = nc.declare_dram_parameter("w", [KP, WCOLS], F32, isOutput=False)
    out = nc.declare_dram_parameter("out", [WCOLS, ROWS], F32, isOutput=True)

    with tile.TileContext(nc) as tc:
        with (
            tc.tile_pool(name="const", bufs=1) as cpool,
            tc.tile_pool(name="sbuf", bufs=4) as pool,
            tc.tile_pool(name="ps", bufs=1, space="PSUM") as psum,
        ):
            # Preload all of W: W_sb[p, k, :] = W[k*128 + p, :]
            w_f32 = cpool.tile([128, NK, WCOLS], F32)
            nc.sync.dma_start(
                out=w_f32[:], in_=w[:].rearrange("(k p) n -> p k n", p=128)
            )
            w_bf = cpool.tile([128, NK, WCOLS], BF16)
            nc.vector.tensor_copy(w_bf[:], w_f32[:])

            acc = [
                psum.tile([WCOLS, sz], F32, name=f"acc{i}", tag=f"acc{i}")
                for i, (_, sz) in enumerate(ROW_SPLITS)
            ]

            for k in range(NK):
                xb = pool.tile([128, ROWS], BF16, tag="xb")
                nc.sync.dma_start(out=xb[:], in_=x_t[k * 128 : (k + 1) * 128, :])
                for i, (o, sz) in enumerate(ROW_SPLITS):
                    nc.tensor.matmul(
                        acc[i][:, :],
                        w_bf[:, k, :],
                        xb[:, o : o + sz],
                        start=(k == 0),
                        stop=(k == NK - 1),
                    )

            res = cpool.tile([WCOLS, ROWS], F32)
            for i, (o, sz) in enumerate(ROW_SPLITS):
                nc.vector.tensor_copy(res[:, o : o + sz], acc[i][:, :])
            nc.sync.dma_start(out=out[:], in_=res[:])

    nc.compile()
    return nc


def _device_matmul(x):
    """Returns A = x @ [Wl1|Wr1|enc_W1[:IN]] as a host-side callable."""
    if "nc" not in _compiled:
        _compiled["nc"] = _build_module()
    return _compiled["nc"]


def _run_device(x, Wcat):
    from concourse import bass_utils

    import ml_dtypes

    nc = _device_matmul(x)
    xT = np.zeros((KP, N), ml_dtypes.bfloat16)
    np.copyto(xT[:IN], x.T, casting="same_kind")
    in_maps = [
        {"x_t": np.ascontiguousarray(xT[:, c * ROWS : (c + 1) * ROWS]), "w": Wcat}
        for c in range(NCORES)
    ]
    res = bass_utils.run_bass_kernel_spmd(nc, in_maps, core_ids=list(range(NCORES)))
    parts = [np.asarray(res.results[c]["out"]) for c in range(NCORES)]
    A = np.concatenate(parts, axis=1).T  # [N, 80]
    return np.ascontiguousarray(A, dtype=np.float32)


def _segment_ops(xl, xr, att, bias, src_s, ds, starts):
    """GATv2 edge attention + aggregation; edge arrays pre-sorted by dst."""
    e = xl[src_s] + xr[ds]                       # [Et, H, C]
    e = np.where(e >= 0, e, NEG * e)
    logits = (e * att[None]).sum(-1)             # [Et, H]
    m = np.maximum.reduceat(logits, starts, axis=0)   # [N, H] (all segs non-empty)
    ea = np.exp(logits - m[ds])
    denom = np.add.reduceat(ea, starts, axis=0)
    alpha = ea / (denom[ds] + np.float32(1e-16))
    contrib = xl[src_s] * alpha[:, :, None]
    seg = np.add.reduceat(contrib, starts, axis=0)    # [N, H, C]
    return seg.reshape(N, H * C) + bias


def kernel(x, edge_index, Wl1, bl1, Wr1, br1, att1, bias1, lin1_W, lin1_b,
           Wl2, bl2, Wr2, br2, att2, bias2, lin2_W, lin2_b,
           enc_W1, enc_b1, enc_W2, enc_b2, enc_W3, enc_b3):
    x = np.asarray(x, np.float32)
    f32 = lambda a: np.asarray(a, np.float32)
    (Wl1, bl1, Wr1, br1, att1, bias1, lin1_W, lin1_b,
     Wl2, bl2, Wr2, br2, att2, bias2, lin2_W, lin2_b,
     enc_W1, enc_b1, enc_W2, enc_b2, enc_W3, enc_b3) = map(
        f32, (Wl1, bl1, Wr1, br1, att1, bias1, lin1_W, lin1_b,
              Wl2, bl2, Wr2, br2, att2, bias2, lin2_W, lin2_b,
              enc_W1, enc_b1, enc_W2, enc_b2, enc_W3, enc_b3))

    # ---- device: fused big matmul ----
    Wcat = np.zeros((KP, WCOLS), np.float32)
    Wcat[:IN, 0:8] = Wl1
    Wcat[:IN, 8:16] = Wr1
    Wcat[:IN, 16:80] = enc_W1[:IN]
    A = _run_device(x, Wcat)            # [N, 80]

    # ---- host: edge prep (self loops, sort by dst) ----
    ei = np.asarray(edge_index).astype(np.int64)
    loop = np.arange(N, dtype=np.int64)
    src = np.concatenate([ei[0], loop])
    dst = np.concatenate([ei[1], loop])
    order = np.argsort(dst, kind="stable")
    src_s = src[order]
    ds = dst[order]
    counts = np.bincount(ds, minlength=N)
    starts = np.zeros(N, np.int64)
    np.cumsum(counts[:-1], out=starts[1:])

    # ---- GAT layer 1 ----
    xl1 = (A[:, 0:8] + bl1).reshape(N, H, C)
    xr1 = (A[:, 8:16] + br1).reshape(N, H, C)
    g1 = _segment_ops(xl1, xr1, att1, bias1, src_s, ds, starts)
    x1 = np.maximum(g1, 0) @ lin1_W + lin1_b          # [N, 1]

    # ---- GAT layer 2 (input is [N,1]) ----
    xl2 = (x1 @ Wl2 + bl2).reshape(N, H, C)
    xr2 = (x1 @ Wr2 + br2).reshape(N, H, C)
    g2 = _segment_ops(xl2, xr2, att2, bias2, src_s, ds, starts)
    x2 = np.maximum(g2, 0) @ lin2_W + lin2_b          # [N, 1]

    # ---- encoder MLP ----
    h = A[:, 16:80] + x1 * enc_W1[IN][None] + x2 * enc_W1[IN + 1][None] + enc_b1
    h = np.maximum(h, 0)
    h = np.maximum(h @ enc_W2 + enc_b2, 0)
    return (h @ enc_W3 + enc_b3).astype(np.float32)



# revision 2
# speedup vs baseline: 1.7569x; 1.7569x over previous
"""Trainium2 kernel for nn_GATv5 (2-layer GATv2 + encoder MLP).

Structure exploited: with xc = concat(x, x1, x2) (x1,x2 are [N,1] GAT outputs),
the only heavy work is the fused matmul  x @ [Wl1 | Wr1 | enc_W1[:IN]]  — a
[10000, 9998] x [9998, 80] product. That runs on 8 NeuronCores, row-sharded
(1250 rows/core, zero collectives). x is shipped row-major in bf16 (cheap
truncating cast on host); the device transposes k-tiles on the fly with the
DMA XBAR (dma_start_transpose) so the host never pays for a 400MB transpose.
The edge-softmax / segment ops (~5 MFLOP on 330k edges) run on host, as do
the tiny [N,8]x[8,1] and [N,64]x[64,32]x[32,1] tails.
"""

import sys
import numpy as np

sys.path.insert(0, "/opt/trn_rl_repo")

N = 10000
IN = 9998
E = 320000
H, C = 2, 4
NEG = 0.2
NCORES = 8
ROWS = N // NCORES          # 1250 valid rows of x per core
ROWS_P = 1280               # padded to a multiple of 16 (XBAR) and 128
NK = 79                     # k-tiles of 128 over the padded contraction dim
KP = NK * 128               # 10112 (IN padded with zeros)
WCOLS = 80                  # 8 (Wl1) + 8 (Wr1) + 64 (enc_W1 cols)
ROW_SPLITS = [(0, 512), (512, 512), (1024, 256)]

_compiled = {}


def _build_module():
    from concourse import bacc, tile, mybir

    F32 = mybir.dt.float32
    BF16 = mybir.dt.bfloat16

    nc = bacc.Bacc(target_bir_lowering=False)
    x_r = nc.declare_dram_parameter("x_r", [ROWS_P, KP], BF16, isOutput=False)
    w = nc.declare_dram_parameter("w", [KP, WCOLS], BF16, isOutput=False)
    out = nc.declare_dram_parameter("out", [WCOLS, ROWS_P], F32, isOutput=True)

    with tile.TileContext(nc) as tc:
        with (
            tc.tile_pool(name="const", bufs=1) as cpool,
            tc.tile_pool(name="sbuf", bufs=4) as pool,
            tc.tile_pool(name="ps", bufs=1, space="PSUM") as psum,
        ):
            # Preload all of W: W_sb[p, k, :] = W[k*128 + p, :]
            w_sb = cpool.tile([128, NK, WCOLS], BF16)
            nc.sync.dma_start(
                out=w_sb[:], in_=w[:].rearrange("(k p) n -> p k n", p=128)
            )

            acc = [
                psum.tile([WCOLS, sz], F32, name=f"acc{i}", tag=f"acc{i}")
                for i, (_, sz) in enumerate(ROW_SPLITS)
            ]

            for k in range(NK):
                # x^T k-tile via DMA XBAR transpose: xT[p, r] = x_r[r, k*128+p]
                xT = pool.tile([128, ROWS_P], BF16, tag="xT")
                nc.sync.dma_start_transpose(xT[:], x_r[:, k * 128 : (k + 1) * 128])
                for i, (o, sz) in enumerate(ROW_SPLITS):
                    nc.tensor.matmul(
                        acc[i][:, :],
                        w_sb[:, k, :],
                        xT[:, o : o + sz],
                        start=(k == 0),
                        stop=(k == NK - 1),
                    )

            res = cpool.tile([WCOLS, ROWS_P], F32)
            for i, (o, sz) in enumerate(ROW_SPLITS):
                nc.vector.tensor_copy(res[:, o : o + sz], acc[i][:, :])
            nc.sync.dma_start(out=out[:], in_=res[:])

    nc.compile()
    return nc


def _get_module():
    if "nc" not in _compiled:
        _compiled["nc"] = _build_module()
    return _compiled["nc"]


def _bf16_trunc_pad(x):
    """x [N, IN] f32 -> [NCORES*ROWS_P, KP] bf16 (truncating cast, zero pad)."""
    import ml_dtypes

    xp = np.zeros((NCORES * ROWS_P, KP), ml_dtypes.bfloat16)
    u16 = x.view(np.uint16)          # little-endian: high half at odd indices
    o16 = xp.view(np.uint16)
    for c in range(NCORES):
        o16[c * ROWS_P : c * ROWS_P + ROWS, :IN] = u16[
            c * ROWS : (c + 1) * ROWS, 1::2
        ]
    return xp


def _run_device(x, Wcat):
    """Returns A = x @ Wcat[:IN] (x f32 [N, IN], Wcat f32 [KP, WCOLS])."""
    from concourse import bass_utils
    import ml_dtypes

    nc = _get_module()
    xp = _bf16_trunc_pad(np.ascontiguousarray(x, np.float32))
    wb = Wcat.astype(ml_dtypes.bfloat16)
    in_maps = [
        {"x_r": xp[c * ROWS_P : (c + 1) * ROWS_P], "w": wb} for c in range(NCORES)
    ]
    res = bass_utils.run_bass_kernel_spmd(nc, in_maps, core_ids=list(range(NCORES)))
    A = np.empty((N, WCOLS), np.float32)
    for c in range(NCORES):
        A[c * ROWS : (c + 1) * ROWS] = np.asarray(res.results[c]["out"])[:, :ROWS].T
    return A


def _segment_ops(xl, xr, att, bias, src_s, ds, starts):
    """GATv2 edge attention + aggregation; edge arrays pre-sorted by dst."""
    e = xl[src_s] + xr[ds]                       # [Et, H, C]
    e = np.where(e >= 0, e, NEG * e)
    logits = (e * att[None]).sum(-1)             # [Et, H]
    m = np.maximum.reduceat(logits, starts, axis=0)   # [N, H] (all segs non-empty)
    ea = np.exp(logits - m[ds])
    denom = np.add.reduceat(ea, starts, axis=0)
    alpha = ea / (denom[ds] + np.float32(1e-16))
    contrib = xl[src_s] * alpha[:, :, None]
    seg = np.add.reduceat(contrib, starts, axis=0)    # [N, H, C]
    return seg.reshape(N, H * C) + bias


def kernel(x, edge_index, Wl1, bl1, Wr1, br1, att1, bias1, lin1_W, lin1_b,
           Wl2, bl2, Wr2, br2, att2, bias2, lin2_W, lin2_b,
           enc_W1, enc_b1, enc_W2, enc_b2, enc_W3, enc_b3):
    x = np.asarray(x, np.float32)
    f32 = lambda a: np.asarray(a, np.float32)
    (Wl1, bl1, Wr1, br1, att1, bias1, lin1_W, lin1_b,
     Wl2, bl2, Wr2, br2, att2, bias2, lin2_W, lin2_b,
     enc_W1, enc_b1, enc_W2, enc_b2, enc_W3, enc_b3) = map(
        f32, (Wl1, bl1, Wr1, br1, att1, bias1, lin1_W, lin1_b,
              Wl2, bl2, Wr2, br2, att2, bias2, lin2_W, lin2_b,
              enc_W1, enc_b1, enc_W2, enc_b2, enc_W3, enc_b3))

    # ---- device: fused big matmul ----
    Wcat = np.zeros((KP, WCOLS), np.float32)
    Wcat[:IN, 0:8] = Wl1
    Wcat[:IN, 8:16] = Wr1
    Wcat[:IN, 16:80] = enc_W1[:IN]
    A = _run_device(x, Wcat)            # [N, 80]

    # ---- host: edge prep (self loops, sort by dst) ----
    ei = np.asarray(edge_index).astype(np.int64)
    loop = np.arange(N, dtype=np.int64)
    src = np.concatenate([ei[0], loop])
    dst = np.concatenate([ei[1], loop])
    order = np.argsort(dst, kind="stable")
    src_s = src[order]
    ds = dst[order]
    counts = np.bincount(ds, minlength=N)
    starts = np.zeros(N, np.int64)
    np.cumsum(counts[:-1], out=starts[1:])

    # ---- GAT layer 1 ----
    xl1 = (A[:, 0:8] + bl1).reshape(N, H, C)
    xr1 = (A[:, 8:16] + br1).reshape(N, H, C)
    g1 = _segment_ops(xl1, xr1, att1, bias1, src_s, ds, starts)
    x1 = np.maximum(g1, 0) @ lin1_W + lin1_b          # [N, 1]

    # ---- GAT layer 2 (input is [N,1]) ----
    xl2 = (x1 @ Wl2 + bl2).reshape(N, H, C)
    xr2 = (x1 @ Wr2 + br2).reshape(N, H, C)
    g2 = _segment_ops(xl2, xr2, att2, bias2, src_s, ds, starts)
    x2 = np.maximum(g2, 0) @ lin2_W + lin2_b          # [N, 1]

    # ---- encoder MLP ----
    h = A[:, 16:80] + x1 * enc_W1[IN][None] + x2 * enc_W1[IN + 1][None] + enc_b1
    h = np.maximum(h, 0)
    h = np.maximum(h @ enc_W2 + enc_b2, 0)
    return (h @ enc_W3 + enc_b3).astype(np.float32)
